# revision 53
# baseline (speedup 1.0000x reference)
"""Causal multi-head attention (RoPE) forward for Trainium2, sharded over 8 NeuronCores.

Problem (hardcoded): B=2, S=2048, E=128, H=16, D=128, inner=2048.
  out = softmax(causal(rope(q@Wq) @ rope(q@Wk).T / sqrt(D))) @ (q@Wv) @ Wo
Sharding: tensor-parallel over heads - core c owns heads {2c, 2c+1} for both
batches (4 attention units/core). Host combines per-head partial outputs.

Design notes (v7):
 - W_o folded into V on the host with a rank-127 SVD: Wv_h@Wo_h = A_h B_h,
   A_h [E,127] on device, B_h [127,E] applied on host (sigma_128 ~ 1e-5 of
   sigma_1). Device AV stationary per chunk = [A-proj | ones] [128tok, 128],
   so PSUM row 127 of the AV accumulation IS the softmax denominator: no den
   tree, no dens DMA (was 21MB/core), no junk memsets.
 - Input DMAs spread across all engine queues so the first rope chain starts
   ~2us in; stage-B emission ordered so the first score matmul's four rope
   dependencies complete first.
 - All matmuls f16. Scores for chunk pairs -> [128,1024] 2-bank PSUM tiles,
   one full-width exp per pair (ACT); diagonal pair computes exact partial
   ranges (two ranged exps). Causal tril masks alternate DVE/gpsimd.
 - Optional Schraudolph exp offload to DVE (f16 bit-trick, 2 tensor_scalar
   ops) for the SCHN[W] leading pairs per window to relieve ACT.
 - rope fold-adds on gpsimd; rope pair-muls, vwo/fin evicts on DVE (last
   window's fins on ACT: its queue is idle at the tail).
 - PSUM: score/stage-B shared pool 3x[128,1024] (6 banks) + 2 avT banks.
 - stage_b(b=1) interleaved into stage_c(b=0) at pair granularity to fill PE
   gaps; AV matmuls lag scores by `lag` pairs so PE never waits on ACT.
"""

import os
import sys
import numpy as np

for _p in ("/root/.axon_site", "/root/.axon_site/_ro/trn_rl_repo",
           "/root/.axon_site/_ro/pypackages", "/opt/trn_rl_repo"):
    if os.path.isdir(_p) and _p not in sys.path:
        sys.path.append(_p)

from contextlib import ExitStack
from itertools import chain

import concourse.bacc as bacc
import concourse.mybir as mybir
import concourse.tile as tile
from concourse import bass_utils

F32 = mybir.dt.float32
F16 = mybir.dt.float16
I16 = mybir.dt.int16
AF = mybir.ActivationFunctionType
ALU = mybir.AluOpType

B, S, E = 2, 2048, 128
H, D = 16, 128
NCORES = 8
HPC = H // NCORES          # heads per core = 2
WIN = 512                  # q-window
NW = S // WIN              # windows per batch = 4
SCALE = 1.0 / np.sqrt(D)

# Schraudolph exp-on-DVE offload: number of leading (non-diagonal) pairs per
# q-window W handled by DVE instead of ACT. Max allowed per window: 2*W.
SCHN = [int(x) for x in os.environ.get("SCHN", "0,0,0,0").split(",")]
SCH_A = float(1024.0 / np.log(2.0)) * SCALE      # I = s*SCH_A + SCH_B
SCH_B = 15.0 * 1024.0 - 59.4

_CACHE = {}


def _build():
    nc = bacc.Bacc("TRN2", target_bir_lowering=False, debug=False)

    qT_d = nc.dram_tensor("qT", [E, B * S], F16, kind="ExternalInput").ap()
    wqk_d = nc.dram_tensor("wqk", [E, 8 * D], F16, kind="ExternalInput").ap()
    wfA_d = nc.dram_tensor("wfA", [E, HPC * E], F16, kind="ExternalInput").ap()
    cs_d = nc.dram_tensor("cs", [D, 2 * S], F16, kind="ExternalInput").ap()
    tril_d = nc.dram_tensor("trilT", [128, 128], F16, kind="ExternalInput").ap()
    fins_d = nc.dram_tensor("fins", [HPC * E, B * S], F16, kind="ExternalOutput").ap()

    with tile.TileContext(nc) as tc, ExitStack() as ctx:
        const = ctx.enter_context(tc.tile_pool(name="const", bufs=1))
        qkp = ctx.enter_context(tc.tile_pool(name="qkp", bufs=1))
        vhp = ctx.enter_context(tc.tile_pool(name="vhp", bufs=1))
        t12p = ctx.enter_context(tc.tile_pool(name="t12p", bufs=6))
        schp = ctx.enter_context(tc.tile_pool(name="schp", bufs=3))
        expp = ctx.enter_context(tc.tile_pool(name="expp", bufs=3))
        finp = ctx.enter_context(tc.tile_pool(name="finp", bufs=6))
        ps_s = ctx.enter_context(tc.tile_pool(name="ps_s", bufs=3, space="PSUM"))
        ps_av = ctx.enter_context(tc.tile_pool(name="ps_av", bufs=2, space="PSUM"))

        # ---- constant loads, spread over engine queues for parallel DMA ----
        wqk_t = const.tile([128, 8 * D], F16, tag="wqk")
        qt_w = [const.tile([128, WIN], F16, tag=f"qt{i}", name=f"qt{i}")
                for i in range(B * NW)]
        cs_t = const.tile([128, 2 * S], F16, tag="cs")
        wfA_t = const.tile([128, HPC * E], F16, tag="wfA")
        tril_t = const.tile([128, 128], F16, tag="trilT")

        # first rope unit (hl0, kind=1) reads wqk cols [256:512]: land first
        nc.sync.dma_start(wqk_t[:, 256:512], wqk_d[:, 256:512])
        nc.sync.dma_start(qt_w[0][:], qT_d[:, 0:WIN])
        nc.sync.dma_start(wqk_t[:, 0:256], wqk_d[:, 0:256])
        nc.sync.dma_start(wqk_t[:, 512:1024], wqk_d[:, 512:1024])
        nc.sync.dma_start(wfA_t[:], wfA_d[:])
        nc.scalar.dma_start(cs_t[:, 0:1024], cs_d[:, 0:1024])
        nc.sync.dma_start(tril_t[:], tril_d[:])
        for i in range(1, B * NW):
            nc.sync.dma_start(qt_w[i][:], qT_d[:, i * WIN:(i + 1) * WIN])
        nc.scalar.dma_start(cs_t[:, 1024:2 * S], cs_d[:, 1024:2 * S])

        # persistent rope'd q/k: (u, kind, w) -> [128, WIN] f16 (feature-major)
        qk = {}
        for u in range(B * HPC):
            for kind in range(2):
                for w in range(NW):
                    qk[(u, kind, w)] = qkp.tile(
                        [128, WIN], F16, tag=f"qk{u}_{kind}_{w}",
                        name=f"qk{u}_{kind}_{w}")
        # persistent [A-proj | ones] stationaries, token-major:
        # vh[b][:, c*256 + hl*128 + (0..126)] = A-projected V, col 127 = 1.0
        vh = {}
        for b in range(B):
            vh[b] = vhp.tile([128, 4096], F16, tag=f"vh{b}", name=f"vh{b}")
            nc.gpsimd.memset(vh[b][:, 127:4096:128], 1.0)

        def _rope_unit(b, w, hl, kind):
            i = b * NW + w
            u = b * HPC + hl
            ja = (hl * 2 + kind) * 256
            ps = ps_s.tile([128, 1024], F32, tag="ps_s",
                           name=f"psb{b}_{w}_{hl}_{kind}")
            nc.tensor.matmul(ps[:, 0:512], wqk_t[:, ja:ja + 128], qt_w[i][:])
            nc.tensor.matmul(ps[:, 512:1024],
                             wqk_t[:, ja + 128:ja + 256], qt_w[i][:])
            t12 = t12p.tile([128, 1024], F16, tag="t12",
                            name=f"t12_{b}_{w}_{hl}_{kind}")
            nc.vector.tensor_mul(
                t12[:], ps[:], cs_t[:, w * 1024:(w + 1) * 1024])
            nc.gpsimd.tensor_add(
                qk[(u, kind, w)][:], t12[:, 0:512], t12[:, 512:1024])

        def _v_unit(b, w):
            i = b * NW + w
            psv = ps_s.tile([128, 1024], F32, tag="ps_s", name=f"psv{b}_{w}")
            for sub in range(4):
                nc.tensor.matmul(psv[:, sub * 256:(sub + 1) * 256],
                                 qt_w[i][:, sub * 128:(sub + 1) * 128],
                                 wfA_t[:])
            # copy A-projection, skipping col 127 of each 128-block (ones col)
            src = psv[:].rearrange("p (s h e) -> p s h e", s=4, e=128)
            dst = vh[b][:, w * 1024:(w + 1) * 1024].rearrange(
                "p (s h e) -> p s h e", s=4, e=128)
            nc.vector.tensor_copy(dst[:, :, :, 0:127], src[:, :, :, 0:127])

        def _r(b, w, kind):
            """Both hl quanta for one (b, w, kind) rope pair."""
            return [(("r", b, w, kind), lambda hl=hl: _rope_unit(b, w, hl, kind))
                    for hl in range(HPC)]

        def _v(b, w):
            return [(("v", b, w), lambda: _v_unit(b, w))]

        def _flush(filler, key):
            """Emit (now) any pending drip quanta the consumer `key` needs."""
            if filler is None:
                return
            keep = []
            for k, fn in filler["drip"]:
                if k == key:
                    fn()
                else:
                    keep.append((k, fn))
            filler["drip"] = keep

        def stage_c(b, W, filler=None, lag=4, tail=False):
            npair = 2 * W + 2
            avs, e2w = {}, {}
            for hl in range(HPC):
                avs[hl] = ps_av.tile([128, WIN], F32, tag="av",
                                     name=f"av{b}_{W}_{hl}")
                e2w[hl] = expp.tile([128, npair * 1024], F16, tag=f"ew{W}",
                                    name=f"e_{b}_{W}_{hl}")
            _flush(filler, ("r", b, W, 0))
            pend_av = []
            order = list(range(npair))
            stop_c = 2 * order[-1] + 1
            for p in order:
                _flush(filler, ("r", b, min(p // 2, W), 1))
                for hl in range(HPC):
                    u = b * HPC + hl
                    # exp-independent PE work first: it executes during any
                    # ps_s-pool wait of the score matmul that follows.
                    if len(pend_av) > lag:
                        _emit_av(b, W, pend_av.pop(0), avs, e2w, filler,
                                 stop_c=stop_c)
                    if filler is not None:
                        st = filler["step"]
                        if (st < 6 or (st % 2 == 0 and st % 10 != 8)) \
                                and filler["drip"]:
                            filler["drip"].pop(0)[1]()
                        filler["step"] = st + 1
                    ps = ps_s.tile([128, 1024], F32, tag="ps_s",
                                   name=f"ps_{b}_{W}_{hl}_{p}")
                    e2 = e2w[hl][:, p * 1024:(p + 1) * 1024]
                    if p < npair - 1:
                        # chunks 2p, 2p+1 full width (odd diag chunk of pair
                        # 2W extended to full half; its junk cols [512:640]
                        # are never read by AV/den)
                        for h2 in range(2):
                            c = 2 * p + h2
                            kw, ks = c // 4, c % 4
                            nc.tensor.matmul(
                                ps[:, h2 * 512:(h2 + 1) * 512],
                                qk[(u, 1, kw)][:, ks * 128:(ks + 1) * 128],
                                qk[(u, 0, W)][:])
                        if p < SCHN[W] and b == 1:
                            t = schp.tile([128, 1024], F16, tag="sch",
                                          name=f"sch_{b}_{W}_{hl}_{p}")
                            nc.vector.tensor_scalar(
                                t[:], ps[:], SCH_A, SCH_B, ALU.mult, ALU.add)
                            nc.vector.tensor_scalar(
                                e2.bitcast(I16), t[:], 0.0, 31743.0,
                                ALU.max, ALU.min)
                        else:
                            nc.scalar.activation(e2, ps[:], AF.Exp,
                                                 scale=float(SCALE))
                        if p == npair - 2:
                            # diag blocks of chunks 4W (cols [0:128]) and
                            # 4W+1 (cols [640:768])
                            nc.gpsimd.tensor_mul(
                                e2[:, 0:128], e2[:, 0:128], tril_t[:])
                            nc.vector.tensor_mul(
                                e2[:, 640:768], e2[:, 640:768], tril_t[:])
                    else:
                        # last pair: exact partial ranges only
                        nc.tensor.matmul(ps[:, 256:512],
                                         qk[(u, 1, W)][:, 256:384],
                                         qk[(u, 0, W)][:, 256:512])
                        nc.tensor.matmul(ps[:, 896:1024],
                                         qk[(u, 1, W)][:, 384:512],
                                         qk[(u, 0, W)][:, 384:512])
                        nc.scalar.activation(e2[:, 256:512], ps[:, 256:512],
                                             AF.Exp, scale=float(SCALE))
                        nc.scalar.activation(e2[:, 896:1024], ps[:, 896:1024],
                                             AF.Exp, scale=float(SCALE))
                        nc.gpsimd.tensor_mul(
                            e2[:, 256:384], e2[:, 256:384], tril_t[:])
                        nc.vector.tensor_mul(
                            e2[:, 896:1024], e2[:, 896:1024], tril_t[:])
                    pend_av.append((hl, p))
            while pend_av:
                _emit_av(b, W, pend_av.pop(0), avs, e2w, filler,
                         stop_c=stop_c)

            for hl in range(HPC):
                # fin eviction: rows 0..126 = unnormalized head output in
                # A-basis, row 127 = softmax denominator.
                fin = finp.tile([128, WIN], F16, tag="fin",
                                name=f"fin{b}_{W}_{hl}")
                if tail and hl == 0:
                    nc.scalar.copy(fin[:], avs[hl][:])
                else:
                    nc.vector.tensor_copy(fin[:], avs[hl][:])
                eng = nc.scalar if (tail and hl == 0) else nc.sync
                eng.dma_start(
                    fins_d[hl * E:(hl + 1) * E,
                           b * S + W * WIN:b * S + (W + 1) * WIN], fin[:])

        def _emit_av(b, W, item, avs, e2w, filler=None, stop_c=None):
            hl, p = item
            _flush(filler, ("v", b, p // 2))
            if stop_c is None:
                stop_c = 4 * W + 3
            for h2 in range(2):
                c = 2 * p + h2
                jlo = max(0, 128 * c - 512 * W)
                nc.tensor.matmul(
                    avs[hl][:, jlo:WIN],
                    vh[b][:, c * 256 + hl * 128:c * 256 + (hl + 1) * 128],
                    e2w[hl][:, p * 1024 + h2 * 512 + jlo:
                            p * 1024 + (h2 + 1) * 512],
                    start=(c == 0), stop=(c == stop_c))

        # Only the first score matmul's four rope dependencies are emitted
        # up front; everything else drips into the stage_c stream (one
        # quantum per pair-step for the first 6 steps, then every other
        # step), ordered earliest-deadline-first so every quantum lands
        # ahead of its first consumer while the PE always has filler work.
        # hl0 units first: the first score matmul's two chains (w0 k-side,
        # w1 q-side, both hl0) head the DVE queue.
        _rope_unit(0, 0, 0, 1)
        _rope_unit(0, 1, 0, 0)
        _rope_unit(0, 0, 1, 1)
        _rope_unit(0, 1, 1, 0)
        drip = (_r(0, 1, 1) + _v(0, 0)
                + _r(0, 2, 0) + _v(0, 1)
                + _r(0, 2, 1) + _v(0, 2)
                + _r(0, 3, 0) + _r(0, 0, 0)
                + _r(0, 3, 1) + _v(0, 3)
                + _r(1, 0, 1) + _r(1, 3, 0) + _r(1, 1, 1)
                + _v(1, 0) + _r(1, 2, 1)
                + _v(1, 1) + _r(1, 3, 1)
                + _v(1, 2) + _r(1, 2, 0)
                + _v(1, 3) + _r(1, 1, 0) + _r(1, 0, 0))
        filler = {"drip": drip, "step": 0}
        for W in [1, 2, 3, 0]:
            stage_c(0, W, filler=filler)
        for W in [3, 2, 1]:
            stage_c(1, W, filler=filler)
        for _, q in filler["drip"]:
            q()
        filler["drip"] = []
        stage_c(1, 0, lag=1, tail=True)

    nc.compile()
    return nc


def _get_nc():
    if "nc" not in _CACHE:
        _CACHE["nc"] = _build()
    return _CACHE["nc"]


def _host_inputs(q):
    """Shared (core-independent) host-side prep."""
    qT = np.ascontiguousarray(q.reshape(B * S, E).T).astype(np.float16)

    half = D // 2
    inv = (1.0 / (10000.0 ** (np.arange(half, dtype=np.float64) * 2.0 / D)))
    ang = np.arange(S, dtype=np.float64)[None, :] * inv[:, None]   # [half, S]
    cosT = np.repeat(np.cos(ang), 2, axis=0)                       # [D, S]
    sinT = np.repeat(np.sin(ang), 2, axis=0)
    cs = np.empty((D, 2 * S), dtype=np.float16)
    for w in range(NW):
        cs[:, w * 1024:w * 1024 + 512] = cosT[:, w * 512:(w + 1) * 512]
        cs[:, w * 1024 + 512:(w + 1) * 1024] = sinT[:, w * 512:(w + 1) * 512]
    # tril[t, j] = 1 if j >= t  (keep q >= t within the diagonal block)
    tril = np.tril(np.ones((128, 128), dtype=np.float16)).T
    tril = np.ascontiguousarray(tril)
    return qT, cs, tril


def _swap_neg(w):
    """W' columns: w2[:, 2i] = -w[:, 2i+1], w2[:, 2i+1] = w[:, 2i]."""
    w2 = np.empty_like(w)
    w2[:, 0::2] = -w[:, 1::2]
    w2[:, 1::2] = w[:, 0::2]
    return w2


def kernel(q, W_q, W_k, W_v, W_o):
    q = np.asarray(q, dtype=np.float32)
    W_q = np.asarray(W_q, dtype=np.float64)
    W_k = np.asarray(W_k, dtype=np.float64)
    W_v = np.asarray(W_v, dtype=np.float64)
    W_o = np.asarray(W_o, dtype=np.float64)

    nc = _get_nc()
    qT, cs, tril = _host_inputs(q)

    in_maps, Bms = [], []
    for c in range(NCORES):
        wqk = np.empty((E, 8 * D), dtype=np.float16)
        wfA = np.zeros((E, HPC * E), dtype=np.float16)
        Bm = np.zeros((HPC, 127, E), dtype=np.float32)
        for hl in range(HPC):
            h = c * HPC + hl
            for kind, Wm in ((0, W_q), (1, W_k)):
                wslc = Wm[:, h * D:(h + 1) * D]
                ja = (hl * 2 + kind) * 256
                wqk[:, ja:ja + D] = wslc
                wqk[:, ja + D:ja + 2 * D] = _swap_neg(wslc)
            wf2 = W_v[:, h * D:(h + 1) * D] @ W_o[h * D:(h + 1) * D, :]
            U, sv, Vt = np.linalg.svd(wf2)
            wfA[:, hl * E:hl * E + 127] = U[:, :127] * sv[:127]
            Bm[hl] = Vt[:127, :]
        in_maps.append({
            "qT": qT, "wqk": wqk, "wfA": wfA, "cs": cs, "trilT": tril,
        })
        Bms.append(Bm)
    _CACHE["Bms"] = Bms

    res = bass_utils.run_bass_kernel_spmd(
        nc, in_maps, core_ids=list(range(NCORES)),
        trace=bool(int(os.environ.get("KERNEL_TRACE", "0"))))
    _CACHE["last_result"] = res

    out = np.zeros((B, S, E), dtype=np.float32)
    for ci, r in enumerate(res.results):
        out += _combine(r, ci)
    return out


def _combine(r, core):
    """Host-side normalization + rank-127 -> E mixing for one core."""
    Bm = _CACHE["Bms"][core]                                  # [hl, 127, E]
    fins = r["fins"].astype(np.float32).reshape(HPC, E, B, S)  # [hl,r,b,q]
    out = np.zeros((B, S, E), dtype=np.float32)
    for hl in range(HPC):
        nrm = fins[hl, :127] / fins[hl, 127][None]            # [127, b, q]
        nrm = nrm.transpose(1, 2, 0).reshape(B * S, 127)
        out += (nrm @ Bm[hl]).reshape(B, S, E)
    return out


# revision 54
# speedup vs baseline: 1.0496x; 1.0496x over previous
"""Causal multi-head attention (RoPE) forward for Trainium2, sharded over 8 NeuronCores.

Problem (hardcoded): B=2, S=2048, E=128, H=16, D=128, inner=2048.
  out = softmax(causal(rope(q@Wq) @ rope(q@Wk).T / sqrt(D))) @ (q@Wv) @ Wo
Sharding: tensor-parallel over heads - core c owns heads {2c, 2c+1} for both
batches (4 attention units/core). Host combines per-head partial outputs.

Design notes (v7):
 - W_o folded into V on the host with a rank-127 SVD: Wv_h@Wo_h = A_h B_h,
   A_h [E,127] on device, B_h [127,E] applied on host (sigma_128 ~ 1e-5 of
   sigma_1). Device AV stationary per chunk = [A-proj | ones] [128tok, 128],
   so PSUM row 127 of the AV accumulation IS the softmax denominator: no den
   tree, no dens DMA (was 21MB/core), no junk memsets.
 - Input DMAs spread across all engine queues so the first rope chain starts
   ~2us in; stage-B emission ordered so the first score matmul's four rope
   dependencies complete first.
 - All matmuls f16. Scores for chunk pairs -> [128,1024] 2-bank PSUM tiles,
   one full-width exp per pair (ACT); diagonal pair computes exact partial
   ranges (two ranged exps). Causal tril masks alternate DVE/gpsimd.
 - Optional Schraudolph exp offload to DVE (f16 bit-trick, 2 tensor_scalar
   ops) for the SCHN[W] leading pairs per window to relieve ACT.
 - rope fold-adds on gpsimd; rope pair-muls, vwo/fin evicts on DVE (last
   window's fins on ACT: its queue is idle at the tail).
 - PSUM: score/stage-B shared pool 3x[128,1024] (6 banks) + 2 avT banks.
 - stage_b(b=1) interleaved into stage_c(b=0) at pair granularity to fill PE
   gaps; AV matmuls lag scores by `lag` pairs so PE never waits on ACT.
"""

import os
import sys
import numpy as np

for _p in ("/root/.axon_site", "/root/.axon_site/_ro/trn_rl_repo",
           "/root/.axon_site/_ro/pypackages", "/opt/trn_rl_repo"):
    if os.path.isdir(_p) and _p not in sys.path:
        sys.path.append(_p)

from contextlib import ExitStack
from itertools import chain

import concourse.bacc as bacc
import concourse.mybir as mybir
import concourse.tile as tile
from concourse import bass_utils

F32 = mybir.dt.float32
F16 = mybir.dt.float16
I16 = mybir.dt.int16
AF = mybir.ActivationFunctionType
ALU = mybir.AluOpType

B, S, E = 2, 2048, 128
H, D = 16, 128
NCORES = 8
HPC = H // NCORES          # heads per core = 2
WIN = 512                  # q-window
NW = S // WIN              # windows per batch = 4
SCALE = 1.0 / np.sqrt(D)

# Schraudolph exp-on-DVE offload: number of leading (non-diagonal) pairs per
# q-window W handled by DVE instead of ACT. Max allowed per window: 2*W.
SCHN = [int(x) for x in os.environ.get("SCHN", "0,0,0,0").split(",")]
SCH_A = float(1024.0 / np.log(2.0)) * SCALE      # I = s*SCH_A + SCH_B
SCH_B = 15.0 * 1024.0 - 59.4

_CACHE = {}


def _build():
    nc = bacc.Bacc("TRN2", target_bir_lowering=False, debug=False)

    qT_d = nc.dram_tensor("qT", [E, B * S], F16, kind="ExternalInput").ap()
    wqk_d = nc.dram_tensor("wqk", [E, 8 * D], F16, kind="ExternalInput").ap()
    wfA_d = nc.dram_tensor("wfA", [E, HPC * E], F16, kind="ExternalInput").ap()
    cs_d = nc.dram_tensor("cs", [D, 2 * S], F16, kind="ExternalInput").ap()
    tril_d = nc.dram_tensor("trilT", [128, 128], F16, kind="ExternalInput").ap()
    fins_d = nc.dram_tensor("fins", [HPC * E, B * S], F16, kind="ExternalOutput").ap()

    with tile.TileContext(nc) as tc, ExitStack() as ctx:
        const = ctx.enter_context(tc.tile_pool(name="const", bufs=1))
        qkp = ctx.enter_context(tc.tile_pool(name="qkp", bufs=1))
        vhp = ctx.enter_context(tc.tile_pool(name="vhp", bufs=1))
        t12p = ctx.enter_context(tc.tile_pool(name="t12p", bufs=6))
        schp = ctx.enter_context(tc.tile_pool(name="schp", bufs=3))
        expp = ctx.enter_context(tc.tile_pool(name="expp", bufs=3))
        finp = ctx.enter_context(tc.tile_pool(name="finp", bufs=6))
        ps_s = ctx.enter_context(tc.tile_pool(name="ps_s", bufs=3, space="PSUM"))
        ps_av = ctx.enter_context(tc.tile_pool(name="ps_av", bufs=2, space="PSUM"))

        # ---- constant loads, spread over engine queues for parallel DMA ----
        wqk_t = const.tile([128, 8 * D], F16, tag="wqk")
        qt_w = [const.tile([128, WIN], F16, tag=f"qt{i}", name=f"qt{i}")
                for i in range(B * NW)]
        cs_t = const.tile([128, 2 * S], F16, tag="cs")
        wfA_t = const.tile([128, HPC * E], F16, tag="wfA")
        tril_t = const.tile([128, 128], F16, tag="trilT")

        # first rope unit (hl0, kind=1) reads wqk cols [256:512]: land first
        nc.sync.dma_start(wqk_t[:, 256:512], wqk_d[:, 256:512])
        nc.sync.dma_start(qt_w[0][:], qT_d[:, 0:WIN])
        nc.sync.dma_start(wqk_t[:, 0:256], wqk_d[:, 0:256])
        nc.sync.dma_start(wqk_t[:, 512:1024], wqk_d[:, 512:1024])
        nc.sync.dma_start(wfA_t[:], wfA_d[:])
        nc.scalar.dma_start(cs_t[:, 0:1024], cs_d[:, 0:1024])
        nc.sync.dma_start(tril_t[:], tril_d[:])
        for i in range(1, B * NW):
            nc.sync.dma_start(qt_w[i][:], qT_d[:, i * WIN:(i + 1) * WIN])
        nc.scalar.dma_start(cs_t[:, 1024:2 * S], cs_d[:, 1024:2 * S])

        # persistent rope'd q/k: (u, kind, w) -> [128, WIN] f16 (feature-major)
        qk = {}
        for u in range(B * HPC):
            for kind in range(2):
                for w in range(NW):
                    qk[(u, kind, w)] = qkp.tile(
                        [128, WIN], F16, tag=f"qk{u}_{kind}_{w}",
                        name=f"qk{u}_{kind}_{w}")
        # persistent [A-proj | ones] stationaries, token-major:
        # vh[b][:, c*256 + hl*128 + (0..126)] = A-projected V, col 127 = 1.0
        vh = {}
        for b in range(B):
            vh[b] = vhp.tile([128, 4096], F16, tag=f"vh{b}", name=f"vh{b}")
            nc.gpsimd.memset(vh[b][:, 127:4096:128], 1.0)

        def _rope_unit(b, w, hl, kind):
            i = b * NW + w
            u = b * HPC + hl
            ja = (hl * 2 + kind) * 256
            ps = ps_s.tile([128, 1024], F32, tag="ps_s",
                           name=f"psb{b}_{w}_{hl}_{kind}")
            nc.tensor.matmul(ps[:, 0:512], wqk_t[:, ja:ja + 128], qt_w[i][:])
            nc.tensor.matmul(ps[:, 512:1024],
                             wqk_t[:, ja + 128:ja + 256], qt_w[i][:])
            t12 = t12p.tile([128, 1024], F16, tag="t12",
                            name=f"t12_{b}_{w}_{hl}_{kind}")
            nc.vector.tensor_mul(
                t12[:], ps[:], cs_t[:, w * 1024:(w + 1) * 1024])
            nc.gpsimd.tensor_add(
                qk[(u, kind, w)][:], t12[:, 0:512], t12[:, 512:1024])

        def _v_unit(b, w):
            i = b * NW + w
            psv = ps_s.tile([128, 1024], F32, tag="ps_s", name=f"psv{b}_{w}")
            for sub in range(4):
                nc.tensor.matmul(psv[:, sub * 256:(sub + 1) * 256],
                                 qt_w[i][:, sub * 128:(sub + 1) * 128],
                                 wfA_t[:])
            # copy A-projection, skipping col 127 of each 128-block (ones col)
            src = psv[:].rearrange("p (s h e) -> p s h e", s=4, e=128)
            dst = vh[b][:, w * 1024:(w + 1) * 1024].rearrange(
                "p (s h e) -> p s h e", s=4, e=128)
            nc.vector.tensor_copy(dst[:, :, :, 0:127], src[:, :, :, 0:127])

        def _r(b, w, kind):
            """Both hl quanta for one (b, w, kind) rope pair."""
            return [(("r", b, w, kind), lambda hl=hl: _rope_unit(b, w, hl, kind))
                    for hl in range(HPC)]

        def _v(b, w):
            return [(("v", b, w), lambda: _v_unit(b, w))]

        def _flush(filler, key):
            """Emit (now) any pending drip quanta the consumer `key` needs."""
            if filler is None:
                return
            keep = []
            for k, fn in filler["drip"]:
                if k == key:
                    fn()
                else:
                    keep.append((k, fn))
            filler["drip"] = keep

        def stage_c(b, W, filler=None, lag=4, tail=False):
            npair = 2 * W + 2
            avs, e2w = {}, {}
            for hl in range(HPC):
                avs[hl] = ps_av.tile([128, WIN], F32, tag="av",
                                     name=f"av{b}_{W}_{hl}")
                e2w[hl] = expp.tile([128, npair * 1024], F16, tag=f"ew{W}",
                                    name=f"e_{b}_{W}_{hl}")
            _flush(filler, ("r", b, W, 0))
            pend_av = []
            order = list(range(npair))
            stop_c = 2 * order[-1] + 1
            for p in order:
                _flush(filler, ("r", b, min(p // 2, W), 1))
                for hl in range(HPC):
                    u = b * HPC + hl
                    # exp-independent PE work first: it executes during any
                    # ps_s-pool wait of the score matmul that follows.
                    if len(pend_av) > lag:
                        _emit_av(b, W, pend_av.pop(0), avs, e2w, filler,
                                 stop_c=stop_c)
                    if filler is not None:
                        st = filler["step"]
                        if (st < 6 or st % 2 == 0) and filler["drip"]:
                            filler["drip"].pop(0)[1]()
                        filler["step"] = st + 1
                    ps = ps_s.tile([128, 1024], F32, tag="ps_s",
                                   name=f"ps_{b}_{W}_{hl}_{p}")
                    e2 = e2w[hl][:, p * 1024:(p + 1) * 1024]
                    if p < npair - 1:
                        # chunks 2p, 2p+1 full width (odd diag chunk of pair
                        # 2W extended to full half; its junk cols [512:640]
                        # are never read by AV/den)
                        for h2 in range(2):
                            c = 2 * p + h2
                            kw, ks = c // 4, c % 4
                            nc.tensor.matmul(
                                ps[:, h2 * 512:(h2 + 1) * 512],
                                qk[(u, 1, kw)][:, ks * 128:(ks + 1) * 128],
                                qk[(u, 0, W)][:])
                        if p < SCHN[W] and b == 1:
                            t = schp.tile([128, 1024], F16, tag="sch",
                                          name=f"sch_{b}_{W}_{hl}_{p}")
                            nc.vector.tensor_scalar(
                                t[:], ps[:], SCH_A, SCH_B, ALU.mult, ALU.add)
                            nc.vector.tensor_scalar(
                                e2.bitcast(I16), t[:], 0.0, 31743.0,
                                ALU.max, ALU.min)
                        else:
                            nc.scalar.activation(e2, ps[:], AF.Exp,
                                                 scale=float(SCALE))
                        if p == npair - 2:
                            # diag blocks of chunks 4W (cols [0:128]) and
                            # 4W+1 (cols [640:768])
                            nc.gpsimd.tensor_mul(
                                e2[:, 0:128], e2[:, 0:128], tril_t[:])
                            nc.vector.tensor_mul(
                                e2[:, 640:768], e2[:, 640:768], tril_t[:])
                    else:
                        # last pair: exact partial ranges only
                        nc.tensor.matmul(ps[:, 256:512],
                                         qk[(u, 1, W)][:, 256:384],
                                         qk[(u, 0, W)][:, 256:512])
                        nc.tensor.matmul(ps[:, 896:1024],
                                         qk[(u, 1, W)][:, 384:512],
                                         qk[(u, 0, W)][:, 384:512])
                        nc.scalar.activation(e2[:, 256:512], ps[:, 256:512],
                                             AF.Exp, scale=float(SCALE))
                        nc.scalar.activation(e2[:, 896:1024], ps[:, 896:1024],
                                             AF.Exp, scale=float(SCALE))
                        nc.gpsimd.tensor_mul(
                            e2[:, 256:384], e2[:, 256:384], tril_t[:])
                        nc.vector.tensor_mul(
                            e2[:, 896:1024], e2[:, 896:1024], tril_t[:])
                    pend_av.append((hl, p))
            while pend_av:
                _emit_av(b, W, pend_av.pop(0), avs, e2w, filler,
                         stop_c=stop_c)

            for hl in range(HPC):
                # fin eviction: rows 0..126 = unnormalized head output in
                # A-basis, row 127 = softmax denominator.
                fin = finp.tile([128, WIN], F16, tag="fin",
                                name=f"fin{b}_{W}_{hl}")
                if tail and hl == 0:
                    nc.scalar.copy(fin[:], avs[hl][:])
                else:
                    nc.vector.tensor_copy(fin[:], avs[hl][:])
                eng = nc.scalar if (tail and hl == 0) else nc.sync
                eng.dma_start(
                    fins_d[hl * E:(hl + 1) * E,
                           b * S + W * WIN:b * S + (W + 1) * WIN], fin[:])

        def _emit_av(b, W, item, avs, e2w, filler=None, stop_c=None):
            hl, p = item
            _flush(filler, ("v", b, p // 2))
            if stop_c is None:
                stop_c = 4 * W + 3
            for h2 in range(2):
                c = 2 * p + h2
                jlo = max(0, 128 * c - 512 * W)
                nc.tensor.matmul(
                    avs[hl][:, jlo:WIN],
                    vh[b][:, c * 256 + hl * 128:c * 256 + (hl + 1) * 128],
                    e2w[hl][:, p * 1024 + h2 * 512 + jlo:
                            p * 1024 + (h2 + 1) * 512],
                    start=(c == 0), stop=(c == stop_c))

        # Only the first score matmul's four rope dependencies are emitted
        # up front; everything else drips into the stage_c stream (one
        # quantum per pair-step for the first 6 steps, then every other
        # step), ordered earliest-deadline-first so every quantum lands
        # ahead of its first consumer while the PE always has filler work.
        # hl0 units first: the first score matmul's two chains (w0 k-side,
        # w1 q-side, both hl0) head the DVE queue.
        _rope_unit(0, 0, 0, 1)
        _rope_unit(0, 1, 0, 0)
        _rope_unit(0, 0, 1, 1)
        _rope_unit(0, 1, 1, 0)
        drip = (_r(0, 1, 1) + _v(0, 0)
                + _r(0, 2, 0) + _v(0, 1)
                + _r(0, 2, 1) + _v(0, 2)
                + _r(0, 3, 0) + _r(0, 0, 0)
                + _r(0, 3, 1) + _v(0, 3)
                + _r(1, 0, 1) + _r(1, 3, 0) + _r(1, 1, 1)
                + _v(1, 0) + _r(1, 2, 1)
                + _v(1, 1) + _r(1, 3, 1)
                + _v(1, 2) + _r(1, 2, 0)
                + _v(1, 3) + _r(1, 1, 0) + _r(1, 0, 0))
        filler = {"drip": drip, "step": 0}
        for W in [1, 2, 3, 0]:
            stage_c(0, W, filler=filler)
        for W in [3, 2, 1]:
            stage_c(1, W, filler=filler)
        for _, q in filler["drip"]:
            q()
        filler["drip"] = []
        stage_c(1, 0, lag=1, tail=True)

    nc.compile()
    return nc


def _get_nc():
    if "nc" not in _CACHE:
        _CACHE["nc"] = _build()
    return _CACHE["nc"]


def _host_inputs(q):
    """Shared (core-independent) host-side prep."""
    qT = np.ascontiguousarray(q.reshape(B * S, E).T).astype(np.float16)

    half = D // 2
    inv = (1.0 / (10000.0 ** (np.arange(half, dtype=np.float64) * 2.0 / D)))
    ang = np.arange(S, dtype=np.float64)[None, :] * inv[:, None]   # [half, S]
    cosT = np.repeat(np.cos(ang), 2, axis=0)                       # [D, S]
    sinT = np.repeat(np.sin(ang), 2, axis=0)
    cs = np.empty((D, 2 * S), dtype=np.float16)
    for w in range(NW):
        cs[:, w * 1024:w * 1024 + 512] = cosT[:, w * 512:(w + 1) * 512]
        cs[:, w * 1024 + 512:(w + 1) * 1024] = sinT[:, w * 512:(w + 1) * 512]
    # tril[t, j] = 1 if j >= t  (keep q >= t within the diagonal block)
    tril = np.tril(np.ones((128, 128), dtype=np.float16)).T
    tril = np.ascontiguousarray(tril)
    return qT, cs, tril


def _swap_neg(w):
    """W' columns: w2[:, 2i] = -w[:, 2i+1], w2[:, 2i+1] = w[:, 2i]."""
    w2 = np.empty_like(w)
    w2[:, 0::2] = -w[:, 1::2]
    w2[:, 1::2] = w[:, 0::2]
    return w2


def kernel(q, W_q, W_k, W_v, W_o):
    q = np.asarray(q, dtype=np.float32)
    W_q = np.asarray(W_q, dtype=np.float64)
    W_k = np.asarray(W_k, dtype=np.float64)
    W_v = np.asarray(W_v, dtype=np.float64)
    W_o = np.asarray(W_o, dtype=np.float64)

    nc = _get_nc()
    qT, cs, tril = _host_inputs(q)

    in_maps, Bms = [], []
    for c in range(NCORES):
        wqk = np.empty((E, 8 * D), dtype=np.float16)
        wfA = np.zeros((E, HPC * E), dtype=np.float16)
        Bm = np.zeros((HPC, 127, E), dtype=np.float32)
        for hl in range(HPC):
            h = c * HPC + hl
            for kind, Wm in ((0, W_q), (1, W_k)):
                wslc = Wm[:, h * D:(h + 1) * D]
                ja = (hl * 2 + kind) * 256
                wqk[:, ja:ja + D] = wslc
                wqk[:, ja + D:ja + 2 * D] = _swap_neg(wslc)
            wf2 = W_v[:, h * D:(h + 1) * D] @ W_o[h * D:(h + 1) * D, :]
            U, sv, Vt = np.linalg.svd(wf2)
            wfA[:, hl * E:hl * E + 127] = U[:, :127] * sv[:127]
            Bm[hl] = Vt[:127, :]
        in_maps.append({
            "qT": qT, "wqk": wqk, "wfA": wfA, "cs": cs, "trilT": tril,
        })
        Bms.append(Bm)
    _CACHE["Bms"] = Bms

    res = bass_utils.run_bass_kernel_spmd(
        nc, in_maps, core_ids=list(range(NCORES)),
        trace=bool(int(os.environ.get("KERNEL_TRACE", "0"))))
    _CACHE["last_result"] = res

    out = np.zeros((B, S, E), dtype=np.float32)
    for ci, r in enumerate(res.results):
        out += _combine(r, ci)
    return out


def _combine(r, core):
    """Host-side normalization + rank-127 -> E mixing for one core."""
    Bm = _CACHE["Bms"][core]                                  # [hl, 127, E]
    fins = r["fins"].astype(np.float32).reshape(HPC, E, B, S)  # [hl,r,b,q]
    out = np.zeros((B, S, E), dtype=np.float32)
    for hl in range(HPC):
        nrm = fins[hl, :127] / fins[hl, 127][None]            # [127, b, q]
        nrm = nrm.transpose(1, 2, 0).reshape(B * S, 127)
        out += (nrm @ Bm[hl]).reshape(B, S, E)
    return out


# revision 55
# speedup vs baseline: 1.0593x; 1.0093x over previous
"""Causal multi-head attention (RoPE) forward for Trainium2, sharded over 8 NeuronCores.

Problem (hardcoded): B=2, S=2048, E=128, H=16, D=128, inner=2048.
  out = softmax(causal(rope(q@Wq) @ rope(q@Wk).T / sqrt(D))) @ (q@Wv) @ Wo
Sharding: tensor-parallel over heads - core c owns heads {2c, 2c+1} for both
batches (4 attention units/core). Host combines per-head partial outputs.

Design notes (v7):
 - W_o folded into V on the host with a rank-127 SVD: Wv_h@Wo_h = A_h B_h,
   A_h [E,127] on device, B_h [127,E] applied on host (sigma_128 ~ 1e-5 of
   sigma_1). Device AV stationary per chunk = [A-proj | ones] [128tok, 128],
   so PSUM row 127 of the AV accumulation IS the softmax denominator: no den
   tree, no dens DMA (was 21MB/core), no junk memsets.
 - Input DMAs spread across all engine queues so the first rope chain starts
   ~2us in; stage-B emission ordered so the first score matmul's four rope
   dependencies complete first.
 - All matmuls f16. Scores for chunk pairs -> [128,1024] 2-bank PSUM tiles,
   one full-width exp per pair (ACT); diagonal pair computes exact partial
   ranges (two ranged exps). Causal tril masks alternate DVE/gpsimd.
 - Optional Schraudolph exp offload to DVE (f16 bit-trick, 2 tensor_scalar
   ops) for the SCHN[W] leading pairs per window to relieve ACT.
 - rope fold-adds on gpsimd; rope pair-muls, vwo/fin evicts on DVE (last
   window's fins on ACT: its queue is idle at the tail).
 - PSUM: score/stage-B shared pool 3x[128,1024] (6 banks) + 2 avT banks.
 - stage_b(b=1) interleaved into stage_c(b=0) at pair granularity to fill PE
   gaps; AV matmuls lag scores by `lag` pairs so PE never waits on ACT.
"""

import os
import sys
import numpy as np

for _p in ("/root/.axon_site", "/root/.axon_site/_ro/trn_rl_repo",
           "/root/.axon_site/_ro/pypackages", "/opt/trn_rl_repo"):
    if os.path.isdir(_p) and _p not in sys.path:
        sys.path.append(_p)

from contextlib import ExitStack
from itertools import chain

import concourse.bacc as bacc
import concourse.mybir as mybir
import concourse.tile as tile
from concourse import bass_utils

F32 = mybir.dt.float32
F16 = mybir.dt.float16
I16 = mybir.dt.int16
AF = mybir.ActivationFunctionType
ALU = mybir.AluOpType

B, S, E = 2, 2048, 128
H, D = 16, 128
NCORES = 8
HPC = H // NCORES          # heads per core = 2
WIN = 512                  # q-window
NW = S // WIN              # windows per batch = 4
SCALE = 1.0 / np.sqrt(D)

# Schraudolph exp-on-DVE offload: number of leading (non-diagonal) pairs per
# q-window W handled by DVE instead of ACT. Max allowed per window: 2*W.
SCHN = [int(x) for x in os.environ.get("SCHN", "0,0,0,0").split(",")]
SCH_A = float(1024.0 / np.log(2.0)) * SCALE      # I = s*SCH_A + SCH_B
SCH_B = 15.0 * 1024.0 - 59.4

_CACHE = {}


def _build():
    nc = bacc.Bacc("TRN2", target_bir_lowering=False, debug=False)

    qT_d = nc.dram_tensor("qT", [E, B * S], F16, kind="ExternalInput").ap()
    wqk_d = nc.dram_tensor("wqk", [E, 8 * D], F16, kind="ExternalInput").ap()
    wfA_d = nc.dram_tensor("wfA", [E, HPC * E], F16, kind="ExternalInput").ap()
    cs_d = nc.dram_tensor("cs", [D, 2 * S], F16, kind="ExternalInput").ap()
    tril_d = nc.dram_tensor("trilT", [128, 128], F16, kind="ExternalInput").ap()
    fins_d = nc.dram_tensor("fins", [HPC * E, B * S], F16, kind="ExternalOutput").ap()

    with tile.TileContext(nc) as tc, ExitStack() as ctx:
        const = ctx.enter_context(tc.tile_pool(name="const", bufs=1))
        qkp = ctx.enter_context(tc.tile_pool(name="qkp", bufs=1))
        vhp = ctx.enter_context(tc.tile_pool(name="vhp", bufs=1))
        t12p = ctx.enter_context(tc.tile_pool(name="t12p", bufs=6))
        schp = ctx.enter_context(tc.tile_pool(name="schp", bufs=3))
        expp = ctx.enter_context(tc.tile_pool(name="expp", bufs=3))
        finp = ctx.enter_context(tc.tile_pool(name="finp", bufs=6))
        ps_s = ctx.enter_context(tc.tile_pool(name="ps_s", bufs=3, space="PSUM"))
        ps_av = ctx.enter_context(tc.tile_pool(name="ps_av", bufs=2, space="PSUM"))

        # ---- constant loads, spread over engine queues for parallel DMA ----
        wqk_t = const.tile([128, 8 * D], F16, tag="wqk")
        qt_w = [const.tile([128, WIN], F16, tag=f"qt{i}", name=f"qt{i}")
                for i in range(B * NW)]
        cs_t = const.tile([128, 2 * S], F16, tag="cs")
        wfA_t = const.tile([128, HPC * E], F16, tag="wfA")
        tril_t = const.tile([128, 128], F16, tag="trilT")

        # DMA order tracks the pre-emitted rope units: unit1 (w0,k1,hl0)
        # needs wqk[256:512]+qt0+cs[0:1024]; unit2 (w1,k0,hl0) needs
        # wqk[0:256]+qt1+cs[1024:2048] (its own transfer so its completion
        # semaphore doesn't wait for w2/w3's cs).
        nc.sync.dma_start(wqk_t[:, 256:512], wqk_d[:, 256:512])
        nc.sync.dma_start(qt_w[0][:], qT_d[:, 0:WIN])
        nc.scalar.dma_start(cs_t[:, 0:1024], cs_d[:, 0:1024])
        nc.sync.dma_start(wqk_t[:, 0:256], wqk_d[:, 0:256])
        nc.sync.dma_start(qt_w[1][:], qT_d[:, WIN:2 * WIN])
        nc.scalar.dma_start(cs_t[:, 1024:2048], cs_d[:, 1024:2048])
        nc.sync.dma_start(wqk_t[:, 512:1024], wqk_d[:, 512:1024])
        nc.sync.dma_start(wfA_t[:], wfA_d[:])
        nc.sync.dma_start(tril_t[:], tril_d[:])
        for i in range(2, B * NW):
            nc.sync.dma_start(qt_w[i][:], qT_d[:, i * WIN:(i + 1) * WIN])
        nc.scalar.dma_start(cs_t[:, 2048:2 * S], cs_d[:, 2048:2 * S])

        # persistent rope'd q/k: (u, kind, w) -> [128, WIN] f16 (feature-major)
        qk = {}
        for u in range(B * HPC):
            for kind in range(2):
                for w in range(NW):
                    qk[(u, kind, w)] = qkp.tile(
                        [128, WIN], F16, tag=f"qk{u}_{kind}_{w}",
                        name=f"qk{u}_{kind}_{w}")
        # persistent [A-proj | ones] stationaries, token-major:
        # vh[b][:, c*256 + hl*128 + (0..126)] = A-projected V, col 127 = 1.0
        vh = {}
        for b in range(B):
            vh[b] = vhp.tile([128, 4096], F16, tag=f"vh{b}", name=f"vh{b}")
            nc.gpsimd.memset(vh[b][:, 127:4096:128], 1.0)

        def _rope_unit(b, w, hl, kind):
            i = b * NW + w
            u = b * HPC + hl
            ja = (hl * 2 + kind) * 256
            ps = ps_s.tile([128, 1024], F32, tag="ps_s",
                           name=f"psb{b}_{w}_{hl}_{kind}")
            nc.tensor.matmul(ps[:, 0:512], wqk_t[:, ja:ja + 128], qt_w[i][:])
            nc.tensor.matmul(ps[:, 512:1024],
                             wqk_t[:, ja + 128:ja + 256], qt_w[i][:])
            t12 = t12p.tile([128, 1024], F16, tag="t12",
                            name=f"t12_{b}_{w}_{hl}_{kind}")
            nc.vector.tensor_mul(
                t12[:], ps[:], cs_t[:, w * 1024:(w + 1) * 1024])
            nc.gpsimd.tensor_add(
                qk[(u, kind, w)][:], t12[:, 0:512], t12[:, 512:1024])

        def _v_unit(b, w):
            i = b * NW + w
            psv = ps_s.tile([128, 1024], F32, tag="ps_s", name=f"psv{b}_{w}")
            for sub in range(4):
                nc.tensor.matmul(psv[:, sub * 256:(sub + 1) * 256],
                                 qt_w[i][:, sub * 128:(sub + 1) * 128],
                                 wfA_t[:])
            # copy A-projection, skipping col 127 of each 128-block (ones col)
            src = psv[:].rearrange("p (s h e) -> p s h e", s=4, e=128)
            dst = vh[b][:, w * 1024:(w + 1) * 1024].rearrange(
                "p (s h e) -> p s h e", s=4, e=128)
            nc.vector.tensor_copy(dst[:, :, :, 0:127], src[:, :, :, 0:127])

        def _r(b, w, kind):
            """Both hl quanta for one (b, w, kind) rope pair."""
            return [(("r", b, w, kind), lambda hl=hl: _rope_unit(b, w, hl, kind))
                    for hl in range(HPC)]

        def _v(b, w):
            return [(("v", b, w), lambda: _v_unit(b, w))]

        def _flush(filler, key):
            """Emit (now) any pending drip quanta the consumer `key` needs."""
            if filler is None:
                return
            keep = []
            for k, fn in filler["drip"]:
                if k == key:
                    fn()
                else:
                    keep.append((k, fn))
            filler["drip"] = keep

        def stage_c(b, W, filler=None, lag=4, tail=False):
            npair = 2 * W + 2
            avs, e2w = {}, {}
            for hl in range(HPC):
                avs[hl] = ps_av.tile([128, WIN], F32, tag="av",
                                     name=f"av{b}_{W}_{hl}")
                e2w[hl] = expp.tile([128, npair * 1024], F16, tag=f"ew{W}",
                                    name=f"e_{b}_{W}_{hl}")
            _flush(filler, ("r", b, W, 0))
            pend_av = []
            order = list(range(npair))
            stop_c = 2 * order[-1] + 1
            for p in order:
                _flush(filler, ("r", b, min(p // 2, W), 1))
                for hl in range(HPC):
                    u = b * HPC + hl
                    # exp-independent PE work first: it executes during any
                    # ps_s-pool wait of the score matmul that follows.
                    if len(pend_av) > lag:
                        _emit_av(b, W, pend_av.pop(0), avs, e2w, filler,
                                 stop_c=stop_c)
                    if filler is not None:
                        st = filler["step"]
                        if (st < 6 or st % 2 == 0) and filler["drip"]:
                            filler["drip"].pop(0)[1]()
                        filler["step"] = st + 1
                    ps = ps_s.tile([128, 1024], F32, tag="ps_s",
                                   name=f"ps_{b}_{W}_{hl}_{p}")
                    e2 = e2w[hl][:, p * 1024:(p + 1) * 1024]
                    if p < npair - 1:
                        # chunks 2p, 2p+1 full width (odd diag chunk of pair
                        # 2W extended to full half; its junk cols [512:640]
                        # are never read by AV/den)
                        for h2 in range(2):
                            c = 2 * p + h2
                            kw, ks = c // 4, c % 4
                            nc.tensor.matmul(
                                ps[:, h2 * 512:(h2 + 1) * 512],
                                qk[(u, 1, kw)][:, ks * 128:(ks + 1) * 128],
                                qk[(u, 0, W)][:])
                        if p < SCHN[W] and b == 1:
                            t = schp.tile([128, 1024], F16, tag="sch",
                                          name=f"sch_{b}_{W}_{hl}_{p}")
                            nc.vector.tensor_scalar(
                                t[:], ps[:], SCH_A, SCH_B, ALU.mult, ALU.add)
                            nc.vector.tensor_scalar(
                                e2.bitcast(I16), t[:], 0.0, 31743.0,
                                ALU.max, ALU.min)
                        else:
                            nc.scalar.activation(e2, ps[:], AF.Exp,
                                                 scale=float(SCALE))
                        if p == npair - 2:
                            # diag blocks of chunks 4W (cols [0:128]) and
                            # 4W+1 (cols [640:768])
                            nc.gpsimd.tensor_mul(
                                e2[:, 0:128], e2[:, 0:128], tril_t[:])
                            nc.vector.tensor_mul(
                                e2[:, 640:768], e2[:, 640:768], tril_t[:])
                    else:
                        # last pair: exact partial ranges only
                        nc.tensor.matmul(ps[:, 256:512],
                                         qk[(u, 1, W)][:, 256:384],
                                         qk[(u, 0, W)][:, 256:512])
                        nc.tensor.matmul(ps[:, 896:1024],
                                         qk[(u, 1, W)][:, 384:512],
                                         qk[(u, 0, W)][:, 384:512])
                        nc.scalar.activation(e2[:, 256:512], ps[:, 256:512],
                                             AF.Exp, scale=float(SCALE))
                        nc.scalar.activation(e2[:, 896:1024], ps[:, 896:1024],
                                             AF.Exp, scale=float(SCALE))
                        nc.gpsimd.tensor_mul(
                            e2[:, 256:384], e2[:, 256:384], tril_t[:])
                        nc.vector.tensor_mul(
                            e2[:, 896:1024], e2[:, 896:1024], tril_t[:])
                    pend_av.append((hl, p))
            while pend_av:
                _emit_av(b, W, pend_av.pop(0), avs, e2w, filler,
                         stop_c=stop_c)

            for hl in range(HPC):
                # fin eviction: rows 0..126 = unnormalized head output in
                # A-basis, row 127 = softmax denominator.
                fin = finp.tile([128, WIN], F16, tag="fin",
                                name=f"fin{b}_{W}_{hl}")
                if tail and hl == 0:
                    nc.scalar.copy(fin[:], avs[hl][:])
                else:
                    nc.vector.tensor_copy(fin[:], avs[hl][:])
                eng = nc.scalar if (tail and hl == 0) else nc.sync
                eng.dma_start(
                    fins_d[hl * E:(hl + 1) * E,
                           b * S + W * WIN:b * S + (W + 1) * WIN], fin[:])

        def _emit_av(b, W, item, avs, e2w, filler=None, stop_c=None):
            hl, p = item
            _flush(filler, ("v", b, p // 2))
            if stop_c is None:
                stop_c = 4 * W + 3
            for h2 in range(2):
                c = 2 * p + h2
                jlo = max(0, 128 * c - 512 * W)
                nc.tensor.matmul(
                    avs[hl][:, jlo:WIN],
                    vh[b][:, c * 256 + hl * 128:c * 256 + (hl + 1) * 128],
                    e2w[hl][:, p * 1024 + h2 * 512 + jlo:
                            p * 1024 + (h2 + 1) * 512],
                    start=(c == 0), stop=(c == stop_c))

        # Only the first score matmul's four rope dependencies are emitted
        # up front; everything else drips into the stage_c stream (one
        # quantum per pair-step for the first 6 steps, then every other
        # step), ordered earliest-deadline-first so every quantum lands
        # ahead of its first consumer while the PE always has filler work.
        # hl0 units first: the first score matmul's two chains (w0 k-side,
        # w1 q-side, both hl0) head the DVE queue.
        _rope_unit(0, 0, 0, 1)
        _rope_unit(0, 1, 0, 0)
        _rope_unit(0, 0, 1, 1)
        _rope_unit(0, 1, 1, 0)
        drip = (_r(0, 1, 1) + _v(0, 0)
                + _r(0, 2, 0) + _v(0, 1)
                + _r(0, 2, 1) + _v(0, 2)
                + _r(0, 3, 0) + _r(0, 0, 0)
                + _r(0, 3, 1) + _v(0, 3)
                + _r(1, 0, 1) + _r(1, 3, 0) + _r(1, 1, 1)
                + _v(1, 0) + _r(1, 2, 1)
                + _v(1, 1) + _r(1, 3, 1)
                + _v(1, 2) + _r(1, 2, 0)
                + _v(1, 3) + _r(1, 1, 0) + _r(1, 0, 0))
        filler = {"drip": drip, "step": 0}
        for W in [1, 2, 3, 0]:
            stage_c(0, W, filler=filler)
        for W in [3, 2, 1]:
            stage_c(1, W, filler=filler)
        for _, q in filler["drip"]:
            q()
        filler["drip"] = []
        stage_c(1, 0, lag=1, tail=True)

    nc.compile()
    return nc


def _get_nc():
    if "nc" not in _CACHE:
        _CACHE["nc"] = _build()
    return _CACHE["nc"]


def _host_inputs(q):
    """Shared (core-independent) host-side prep."""
    qT = np.ascontiguousarray(q.reshape(B * S, E).T).astype(np.float16)

    half = D // 2
    inv = (1.0 / (10000.0 ** (np.arange(half, dtype=np.float64) * 2.0 / D)))
    ang = np.arange(S, dtype=np.float64)[None, :] * inv[:, None]   # [half, S]
    cosT = np.repeat(np.cos(ang), 2, axis=0)                       # [D, S]
    sinT = np.repeat(np.sin(ang), 2, axis=0)
    cs = np.empty((D, 2 * S), dtype=np.float16)
    for w in range(NW):
        cs[:, w * 1024:w * 1024 + 512] = cosT[:, w * 512:(w + 1) * 512]
        cs[:, w * 1024 + 512:(w + 1) * 1024] = sinT[:, w * 512:(w + 1) * 512]
    # tril[t, j] = 1 if j >= t  (keep q >= t within the diagonal block)
    tril = np.tril(np.ones((128, 128), dtype=np.float16)).T
    tril = np.ascontiguousarray(tril)
    return qT, cs, tril


def _swap_neg(w):
    """W' columns: w2[:, 2i] = -w[:, 2i+1], w2[:, 2i+1] = w[:, 2i]."""
    w2 = np.empty_like(w)
    w2[:, 0::2] = -w[:, 1::2]
    w2[:, 1::2] = w[:, 0::2]
    return w2


def kernel(q, W_q, W_k, W_v, W_o):
    q = np.asarray(q, dtype=np.float32)
    W_q = np.asarray(W_q, dtype=np.float64)
    W_k = np.asarray(W_k, dtype=np.float64)
    W_v = np.asarray(W_v, dtype=np.float64)
    W_o = np.asarray(W_o, dtype=np.float64)

    nc = _get_nc()
    qT, cs, tril = _host_inputs(q)

    in_maps, Bms = [], []
    for c in range(NCORES):
        wqk = np.empty((E, 8 * D), dtype=np.float16)
        wfA = np.zeros((E, HPC * E), dtype=np.float16)
        Bm = np.zeros((HPC, 127, E), dtype=np.float32)
        for hl in range(HPC):
            h = c * HPC + hl
            for kind, Wm in ((0, W_q), (1, W_k)):
                wslc = Wm[:, h * D:(h + 1) * D]
                ja = (hl * 2 + kind) * 256
                wqk[:, ja:ja + D] = wslc
                wqk[:, ja + D:ja + 2 * D] = _swap_neg(wslc)
            wf2 = W_v[:, h * D:(h + 1) * D] @ W_o[h * D:(h + 1) * D, :]
            U, sv, Vt = np.linalg.svd(wf2)
            wfA[:, hl * E:hl * E + 127] = U[:, :127] * sv[:127]
            Bm[hl] = Vt[:127, :]
        in_maps.append({
            "qT": qT, "wqk": wqk, "wfA": wfA, "cs": cs, "trilT": tril,
        })
        Bms.append(Bm)
    _CACHE["Bms"] = Bms

    res = bass_utils.run_bass_kernel_spmd(
        nc, in_maps, core_ids=list(range(NCORES)),
        trace=bool(int(os.environ.get("KERNEL_TRACE", "0"))))
    _CACHE["last_result"] = res

    out = np.zeros((B, S, E), dtype=np.float32)
    for ci, r in enumerate(res.results):
        out += _combine(r, ci)
    return out


def _combine(r, core):
    """Host-side normalization + rank-127 -> E mixing for one core."""
    Bm = _CACHE["Bms"][core]                                  # [hl, 127, E]
    fins = r["fins"].astype(np.float32).reshape(HPC, E, B, S)  # [hl,r,b,q]
    out = np.zeros((B, S, E), dtype=np.float32)
    for hl in range(HPC):
        nrm = fins[hl, :127] / fins[hl, 127][None]            # [127, b, q]
        nrm = nrm.transpose(1, 2, 0).reshape(B * S, 127)
        out += (nrm @ Bm[hl]).reshape(B, S, E)
    return out
